# revision 35
# baseline (speedup 1.0000x reference)
"""MoE top-2 routing kernel for Trainium2 (8 NeuronCores, expert-parallel).

Problem: nn_CoPRIMEModel_21861383537419 (moe_routing).
  x: (4, 2048, 1024), gate_W: (8, 1024), W1: (8, 1024, 4096), b1: (8, 4096),
  W2: (8, 4096, 1024), b2: (8, 1024).  Top-2 of 8 experts, exact GELU.

Strategy (expert parallel, per the sharding hint): the host computes the gate
(softmax + top-2) to decide token placement AND the per-(token, expert)
combine weight w, gathers each expert's tokens, and ships expert e's weights,
its gathered tokens, and the w row to core e.  Each core runs the dense
expert MLP
  y = w * (gelu(x @ W1 + b1) @ W2 + b2)
on the TensorEngine (bf16 operands, fp32 PSUM accumulation) and returns y^T.
The host scatter-adds the two pre-scaled per-expert contributions back into
token order.

Device layouts (tokens always the matmul free dim):
  mm1: psum[m,s] += W1[d,m]-as-lhsT . xgT[d,s]   (stationary = W1 tile)
  mm2: psum[d,s] += W2[m,d]-as-lhsT . hT[m,s]    (stationary = W2 tile)

Tokens are processed in groups (<=1056 tokens, 2-3 chunks of <=512): hT for a
whole group stays in SBUF, W1/W2 stream from HBM once per group.  mm2 keeps
each 256-wide W2 column block resident across chunks so PSUM stays within
4 + 4 banks.
"""

import numpy as np
import ml_dtypes
from contextlib import ExitStack

import concourse.bacc as bacc
import concourse.mybir as mybir
import concourse.tile as tile

B, S, D, M, E = 4, 2048, 1024, 4096, 8
P = 128
DT = D // P   # 8 d-tiles
MT = M // P   # 32 m-tiles
F32 = mybir.dt.float32
BF16 = mybir.dt.bfloat16
NP_BF16 = ml_dtypes.bfloat16

MAXG = 1056   # max tokens per group (hT resident per group)


def _groups(cap):
    """Split cap into groups of <=MAXG, multiples of 32.

    The first group is small (512) so the initial x feed is short and the PE
    starts sooner; later groups hide their loads under earlier compute.  The
    remainder rides with the last group so no group is ever DMA-bound.
    """
    sizes = [min(512, cap)]
    rem = cap - sizes[0]
    while rem > MAXG:
        sizes.append(1024)
        rem -= 1024
    if rem:
        sizes.append(rem)
    assert sum(sizes) == cap and all(0 < g <= MAXG for g in sizes)
    out, off = [], 0
    for g in sizes:
        out.append((off, g))
        off += g
    return out


def _chunks(g):
    cs, off = [], 0
    while off < g:
        c = min(512, g - off)
        cs.append((off, c))
        off += c
    return cs


def build_nc(cap):
    """Build (and bacc-compile) the per-core SPMD expert kernel for capacity cap."""
    assert cap % 32 == 0
    nc = bacc.Bacc(
        "TRN2",
        target_bir_lowering=False,
        debug=False,
        enable_asserts=False,
        num_devices=1,
    )
    xgT = nc.dram_tensor("xgT", [D, cap], BF16, kind="ExternalInput").ap()
    w1 = nc.dram_tensor("w1", [D, M], BF16, kind="ExternalInput").ap()
    w2 = nc.dram_tensor("w2", [M, D], BF16, kind="ExternalInput").ap()
    b1v = nc.dram_tensor("b1v", [M], F32, kind="ExternalInput").ap()
    b2v = nc.dram_tensor("b2v", [D], F32, kind="ExternalInput").ap()
    wrep = nc.dram_tensor("wrep", [P, cap], BF16, kind="ExternalInput").ap()
    yT = nc.dram_tensor("yT", [D, cap], F32, kind="ExternalOutput").ap()

    AF = mybir.ActivationFunctionType
    ALU = mybir.AluOpType

    with tile.TileContext(nc) as tc, ExitStack() as ctx:
        const = ctx.enter_context(tc.tile_pool(name="const", bufs=1))
        xg_pool = ctx.enter_context(tc.tile_pool(name="xg", bufs=2 * DT))
        h_pool = ctx.enter_context(tc.tile_pool(name="h", bufs=MT))
        w1_pool = ctx.enter_context(tc.tile_pool(name="w1", bufs=3))
        w2_pool = ctx.enter_context(tc.tile_pool(name="w2", bufs=2))
        y_pool = ctx.enter_context(tc.tile_pool(name="y", bufs=3))
        ps1 = ctx.enter_context(tc.tile_pool(name="ps1", bufs=4, space="PSUM"))
        ps2 = ctx.enter_context(tc.tile_pool(name="ps2", bufs=4, space="PSUM"))

        w1r = w1.rearrange("(dt p) m -> p dt m", p=P)
        w2r = w2.rearrange("(mt p) d -> p mt d", p=P)

        def issue_w2(dtp):
            t = w2_pool.tile([P, MT, 2 * P], BF16, tag="w2")
            nc.sync.dma_start(
                t[:], w2r[:, :, dtp * 2 * P : (dtp + 1) * 2 * P]
            )
            return t

        b1_sb = b2_sb = wb_sb = None

        for gi, (g0, G) in enumerate(_groups(cap)):
            chunks = _chunks(G)

            # --- load this group's tokens (xgT columns g0:g0+G) ---
            # First group: chunk-0 slabs and the first w1 granule go first so
            # the PE can start ~2us in; constants trail the critical DMAs.
            xg = [
                xg_pool.tile([P, MAXG], BF16, tag="xg", name=f"xg{dt}")
                for dt in range(DT)
            ]

            def issue_w1(k, eng=None):
                t = w1_pool.tile([P, DT, 2 * P], BF16, tag="w1")
                (eng or nc.sync).dma_start(
                    t[:], w1r[:, :, 2 * k * P : 2 * (k + 1) * P]
                )
                return t

            w1_tiles = {}
            if gi == 0:
                # b1 rides the (otherwise idle) SWDGE ring so it lands early
                # without occupying the HWDGE stream; the first activation
                # (and thus PSUM recycling) depends on it.
                b1_sb = const.tile([P, MT], F32, tag="b1")
                nc.gpsimd.dma_start(b1_sb[:], b1v.rearrange("(t p) -> p t", p=P))
                w1_tiles[0] = issue_w1(0)
            for dt in range(DT):
                nc.sync.dma_start(
                    xg[dt][:, :G], xgT[dt * P : (dt + 1) * P, g0 : g0 + G]
                )
            if gi == 0:
                w1_tiles[1] = issue_w1(1)

            w2_tiles = {}

            # --- mm1: hT[m,s] = gelu(sum_d W1[d,m]^T xg[d,s] + b1[m]) ---
            h_tiles = []
            w1t = None
            for mt in range(MT):
                if mt % 2 == 0:
                    k = mt // 2
                    w1t = w1_tiles.pop(k, None) or issue_w1(k)
                    # Stay two granules ahead of the PE (bufs=3).
                    for ka in (k + 1, k + 2):
                        if ka < MT // 2 and ka not in w1_tiles:
                            w1_tiles[ka] = issue_w1(ka)
                ht = h_pool.tile([P, MAXG], BF16, tag="h")
                # Chunk-major so each chunk's PSUM accumulation completes (and
                # is released by its activation) as early as possible.
                for ci, (c0, cw) in enumerate(chunks):
                    hps = ps1.tile([P, 512], F32, tag="ps1")
                    for dt in range(DT):
                        lhs = w1t[:, dt, (mt % 2) * P : (mt % 2 + 1) * P]
                        nc.tensor.matmul(
                            hps[:, :cw],
                            lhs,
                            xg[dt][:, c0 : c0 + cw],
                            start=(dt == 0),
                            stop=(dt == DT - 1),
                        )
                    nc.scalar.activation(
                        ht[:, c0 : c0 + cw],
                        hps[:, :cw],
                        AF.Gelu,
                        bias=b1_sb[:, mt : mt + 1],
                    )
                h_tiles.append(ht)
                # Pre-issue the first two W2 column blocks mid-mm1: late
                # enough not to contend with the w1 stream, early enough to
                # land before mm2 starts.
                if mt == 16:
                    w2_tiles[0] = issue_w2(0)
                    if gi == 0:
                        b2_sb = const.tile([P, DT], F32, tag="b2")
                        nc.sync.dma_start(
                            b2_sb[:], b2v.rearrange("(t p) -> p t", p=P)
                        )
                        wb_sb = const.tile([P, cap], BF16, tag="wb")
                        nc.sync.dma_start(wb_sb[:], wrep[:])
                elif mt == 24:
                    w2_tiles[1] = issue_w2(1)

            # --- mm2: y[d,s] = (sum_m W2[m,d] hT[m,s] + b2[d]) * w ---
            # W2 column block (256 d-cols, all 32 m-slabs) resident per dtp.
            yTr = yT.rearrange("(dt p) s -> p dt s", p=P)
            last_group = g0 + G == cap
            for dtp in range(DT // 2):
                w2t = w2_tiles.pop(dtp)
                mm2_chunks = chunks
                if last_group and dtp == DT // 2 - 1:
                    # Final d-pair: split the tail (last ~600 cols) into
                    # 128-col pieces so each piece's DVE+store drain hides
                    # under the next piece's matmuls instead of dangling
                    # after the very last one.
                    mm2_chunks = list(chunks[:-2])
                    for c0l, cwl in chunks[-2:]:
                        mm2_chunks += [
                            (c0l + o, min(128, cwl - o))
                            for o in range(0, cwl, 128)
                        ]
                for ci, (c0, cw) in enumerate(mm2_chunks):
                    ye = y_pool.tile([P, 2, 512], F32, tag="y")
                    for dj in range(2):
                        yps = ps2.tile([P, 512], F32, tag="ps2")
                        for mt in range(MT):
                            nc.tensor.matmul(
                                yps[:, :cw],
                                w2t[:, mt, dj * P : (dj + 1) * P],
                                h_tiles[mt][:, c0 : c0 + cw],
                                start=(mt == 0),
                                stop=(mt == MT - 1),
                            )
                        dt = dtp * 2 + dj
                        nc.vector.tensor_scalar(
                            ye[:, dj, :cw],
                            yps[:, :cw],
                            b2_sb[:, dt : dt + 1],
                            None,
                            op0=ALU.add,
                        )
                        nc.vector.tensor_mul(
                            ye[:, dj, :cw],
                            ye[:, dj, :cw],
                            wb_sb[:, g0 + c0 : g0 + c0 + cw],
                        )
                    # One merged store for both d-tiles, issued from the
                    # ACT queue (idle during mm2) off the SP ring.  In the
                    # final d-pair the loads are done, so alternate with the
                    # idle SP ring to keep store *dispatches* off the
                    # end-of-kernel critical path.
                    if mm2_chunks is not chunks and ci % 2 == 0:
                        store_eng = nc.sync
                    else:
                        store_eng = nc.scalar
                    store_eng.dma_start(
                        yTr[
                            :,
                            dtp * 2 : dtp * 2 + 2,
                            g0 + c0 : g0 + c0 + cw,
                        ],
                        ye[:, :, :cw],
                    )
                if dtp + 2 < DT // 2:
                    w2_tiles[dtp + 2] = issue_w2(dtp + 2)

    nc.compile()
    return nc


_nc_cache = {}


def _get_nc(cap):
    if cap not in _nc_cache:
        _nc_cache[cap] = build_nc(cap)
    return _nc_cache[cap]


def host_route(xf, gate_W):
    """Host gate: top-2 expert indices + normalized combine weights."""
    logits = xf @ gate_W.T.astype(np.float32)
    gmax = logits.max(axis=1, keepdims=True)
    gexp = np.exp(logits - gmax)
    gate = gexp / gexp.sum(axis=1, keepdims=True)
    top2 = np.argpartition(gate, E - 2, axis=1)[:, E - 2 :]
    tw = np.take_along_axis(gate, top2, axis=1)
    tw = tw / (tw.sum(axis=1, keepdims=True) + 1e-9)
    idx, wsel = [], []
    for e in range(E):
        hit = top2 == e
        rows = np.nonzero(hit.any(axis=1))[0]
        w_e = (np.take_along_axis(tw, hit.argmax(axis=1)[:, None], axis=1))[
            rows, 0
        ]
        idx.append(rows)
        wsel.append(w_e.astype(np.float32))
    return idx, wsel


def make_in_maps(xf, W1, b1, W2, b2, idx, wsel, cap):
    in_maps = []
    for e in range(E):
        xg = np.zeros((D, cap), NP_BF16)
        ne = len(idx[e])
        xg[:, :ne] = xf[idx[e]].T.astype(NP_BF16)
        wrow = np.zeros((cap,), NP_BF16)
        wrow[:ne] = wsel[e].astype(NP_BF16)
        wrep = np.ascontiguousarray(np.broadcast_to(wrow, (P, cap)))
        in_maps.append(
            {
                "xgT": xg,
                "w1": np.ascontiguousarray(W1[e]).astype(NP_BF16),
                "w2": np.ascontiguousarray(W2[e]).astype(NP_BF16),
                "b1v": np.ascontiguousarray(b1[e]),
                "b2v": np.ascontiguousarray(b2[e]),
                "wrep": wrep,
            }
        )
    return in_maps


def kernel(**inputs):
    from concourse.bass_utils import run_bass_kernel_spmd

    x = np.asarray(inputs["x"], dtype=np.float32)
    gate_W = np.asarray(inputs["gate_W"], dtype=np.float32)
    W1 = np.asarray(inputs["W1"], dtype=np.float32)
    b1 = np.asarray(inputs["b1"], dtype=np.float32)
    W2 = np.asarray(inputs["W2"], dtype=np.float32)
    b2 = np.asarray(inputs["b2"], dtype=np.float32)

    Bs, Ss, Ds = x.shape
    xf = np.ascontiguousarray(x.reshape(-1, Ds))
    idx, wsel = host_route(xf, gate_W)
    cap = max(P, -(-max(len(i) for i in idx) // 32) * 32)

    nc = _get_nc(cap)
    in_maps = make_in_maps(xf, W1, b1, W2, b2, idx, wsel, cap)
    res = run_bass_kernel_spmd(nc, in_maps, core_ids=list(range(E)))

    out = np.zeros_like(xf)
    for e in range(E):
        yTe = res.results[e]["yT"]  # [D, cap]
        ne = len(idx[e])
        out[idx[e]] += yTe[:, :ne].T
    return out.reshape(Bs, Ss, Ds)


# revision 37
# speedup vs baseline: 1.0049x; 1.0049x over previous
"""MoE top-2 routing kernel for Trainium2 (8 NeuronCores, expert-parallel).

Problem: nn_CoPRIMEModel_21861383537419 (moe_routing).
  x: (4, 2048, 1024), gate_W: (8, 1024), W1: (8, 1024, 4096), b1: (8, 4096),
  W2: (8, 4096, 1024), b2: (8, 1024).  Top-2 of 8 experts, exact GELU.

Strategy (expert parallel, per the sharding hint): the host computes the gate
(softmax + top-2) to decide token placement AND the per-(token, expert)
combine weight w, gathers each expert's tokens, and ships expert e's weights,
its gathered tokens, and the w row to core e.  Each core runs the dense
expert MLP
  y = w * (gelu(x @ W1 + b1) @ W2 + b2)
on the TensorEngine (bf16 operands, fp32 PSUM accumulation) and returns y^T.
The host scatter-adds the two pre-scaled per-expert contributions back into
token order.

Device layouts (tokens always the matmul free dim):
  mm1: psum[m,s] += W1[d,m]-as-lhsT . xgT[d,s]   (stationary = W1 tile)
  mm2: psum[d,s] += W2[m,d]-as-lhsT . hT[m,s]    (stationary = W2 tile)

Tokens are processed in groups (<=1056 tokens, 2-3 chunks of <=512): hT for a
whole group stays in SBUF, W1/W2 stream from HBM once per group.  mm2 keeps
each 256-wide W2 column block resident across chunks so PSUM stays within
4 + 4 banks.
"""

import numpy as np
import ml_dtypes
from contextlib import ExitStack

import concourse.bacc as bacc
import concourse.mybir as mybir
import concourse.tile as tile

B, S, D, M, E = 4, 2048, 1024, 4096, 8
P = 128
DT = D // P   # 8 d-tiles
MT = M // P   # 32 m-tiles
F32 = mybir.dt.float32
BF16 = mybir.dt.bfloat16
NP_BF16 = ml_dtypes.bfloat16

MAXG = 1056   # max tokens per group (hT resident per group)


def _groups(cap):
    """Split cap into groups of <=MAXG, multiples of 32.

    The first group is small (512) so the initial x feed is short and the PE
    starts sooner; later groups hide their loads under earlier compute.  The
    remainder rides with the last group so no group is ever DMA-bound.
    """
    sizes = [min(512, cap)]
    rem = cap - sizes[0]
    while rem > MAXG:
        sizes.append(1024)
        rem -= 1024
    if rem:
        sizes.append(rem)
    assert sum(sizes) == cap and all(0 < g <= MAXG for g in sizes)
    out, off = [], 0
    for g in sizes:
        out.append((off, g))
        off += g
    return out


def _chunks(g):
    cs, off = [], 0
    while off < g:
        c = min(512, g - off)
        cs.append((off, c))
        off += c
    return cs


def build_nc(cap):
    """Build (and bacc-compile) the per-core SPMD expert kernel for capacity cap."""
    assert cap % 32 == 0
    nc = bacc.Bacc(
        "TRN2",
        target_bir_lowering=False,
        debug=False,
        enable_asserts=False,
        num_devices=1,
    )
    xgT = nc.dram_tensor("xgT", [D, cap], BF16, kind="ExternalInput").ap()
    w1 = nc.dram_tensor("w1", [D, M], BF16, kind="ExternalInput").ap()
    w2 = nc.dram_tensor("w2", [M, D], BF16, kind="ExternalInput").ap()
    b1v = nc.dram_tensor("b1v", [M], F32, kind="ExternalInput").ap()
    b2v = nc.dram_tensor("b2v", [D], F32, kind="ExternalInput").ap()
    wrep = nc.dram_tensor("wrep", [P, cap], BF16, kind="ExternalInput").ap()
    yT = nc.dram_tensor("yT", [D, cap], F32, kind="ExternalOutput").ap()

    AF = mybir.ActivationFunctionType
    ALU = mybir.AluOpType

    with tile.TileContext(nc) as tc, ExitStack() as ctx:
        const = ctx.enter_context(tc.tile_pool(name="const", bufs=1))
        xg_pool = ctx.enter_context(tc.tile_pool(name="xg", bufs=2 * DT))
        h_pool = ctx.enter_context(tc.tile_pool(name="h", bufs=MT))
        w1_pool = ctx.enter_context(tc.tile_pool(name="w1", bufs=3))
        w2_pool = ctx.enter_context(tc.tile_pool(name="w2", bufs=2))
        y_pool = ctx.enter_context(tc.tile_pool(name="y", bufs=3))
        ps1 = ctx.enter_context(tc.tile_pool(name="ps1", bufs=4, space="PSUM"))
        ps2 = ctx.enter_context(tc.tile_pool(name="ps2", bufs=4, space="PSUM"))

        w1r = w1.rearrange("(dt p) m -> p dt m", p=P)
        w2r = w2.rearrange("(mt p) d -> p mt d", p=P)

        def issue_w2(dtp):
            t = w2_pool.tile([P, MT, 2 * P], BF16, tag="w2")
            nc.sync.dma_start(
                t[:], w2r[:, :, dtp * 2 * P : (dtp + 1) * 2 * P]
            )
            return t

        b1_sb = b2_sb = wb_sb = None

        for gi, (g0, G) in enumerate(_groups(cap)):
            chunks = _chunks(G)

            # --- load this group's tokens (xgT columns g0:g0+G) ---
            # First group: chunk-0 slabs and the first w1 granule go first so
            # the PE can start ~2us in; constants trail the critical DMAs.
            xg = [
                xg_pool.tile([P, MAXG], BF16, tag="xg", name=f"xg{dt}")
                for dt in range(DT)
            ]

            def issue_w1(k, eng=None):
                t = w1_pool.tile([P, DT, 2 * P], BF16, tag="w1")
                (eng or nc.sync).dma_start(
                    t[:], w1r[:, :, 2 * k * P : 2 * (k + 1) * P]
                )
                return t

            w1_tiles = {}
            if gi == 0:
                # b1 rides the (otherwise idle) SWDGE ring so it lands early
                # without occupying the HWDGE stream; the first activation
                # (and thus PSUM recycling) depends on it.
                b1_sb = const.tile([P, MT], F32, tag="b1")
                nc.gpsimd.dma_start(b1_sb[:], b1v.rearrange("(t p) -> p t", p=P))
                w1_tiles[0] = issue_w1(0)
            for dt in range(DT):
                nc.sync.dma_start(
                    xg[dt][:, :G], xgT[dt * P : (dt + 1) * P, g0 : g0 + G]
                )
            if gi == 0:
                w1_tiles[1] = issue_w1(1)

            w2_tiles = {}

            # --- mm1: hT[m,s] = gelu(sum_d W1[d,m]^T xg[d,s] + b1[m]) ---
            h_tiles = []
            w1t = None
            for mt in range(MT):
                if mt % 2 == 0:
                    k = mt // 2
                    w1t = w1_tiles.pop(k, None) or issue_w1(k)
                    # Stay two granules ahead of the PE (bufs=3).
                    for ka in (k + 1, k + 2):
                        if ka < MT // 2 and ka not in w1_tiles:
                            w1_tiles[ka] = issue_w1(ka)
                ht = h_pool.tile([P, MAXG], BF16, tag="h")
                # Chunk-major so each chunk's PSUM accumulation completes (and
                # is released by its activation) as early as possible.
                for ci, (c0, cw) in enumerate(chunks):
                    hps = ps1.tile([P, 512], F32, tag="ps1")
                    for dt in range(DT):
                        lhs = w1t[:, dt, (mt % 2) * P : (mt % 2 + 1) * P]
                        nc.tensor.matmul(
                            hps[:, :cw],
                            lhs,
                            xg[dt][:, c0 : c0 + cw],
                            start=(dt == 0),
                            stop=(dt == DT - 1),
                        )
                    nc.scalar.activation(
                        ht[:, c0 : c0 + cw],
                        hps[:, :cw],
                        AF.Gelu,
                        bias=b1_sb[:, mt : mt + 1],
                    )
                h_tiles.append(ht)
                # Pre-issue the first two W2 column blocks mid-mm1: late
                # enough not to contend with the w1 stream, early enough to
                # land before mm2 starts.
                if mt == 16:
                    w2_tiles[0] = issue_w2(0)
                    if gi == 0:
                        b2_sb = const.tile([P, DT], F32, tag="b2")
                        nc.sync.dma_start(
                            b2_sb[:], b2v.rearrange("(t p) -> p t", p=P)
                        )
                        wb_sb = const.tile([P, cap], BF16, tag="wb")
                        nc.sync.dma_start(wb_sb[:], wrep[:])
                elif mt == 24:
                    w2_tiles[1] = issue_w2(1)

            # --- mm2: y[d,s] = (sum_m W2[m,d] hT[m,s] + b2[d]) * w ---
            # W2 column block (256 d-cols, all 32 m-slabs) resident per dtp.
            yTr = yT.rearrange("(dt p) s -> p dt s", p=P)
            last_group = g0 + G == cap
            for dtp in range(DT // 2):
                w2t = w2_tiles.pop(dtp)
                mm2_chunks = chunks
                if last_group and dtp == DT // 2 - 1:
                    # Final d-pair: split the tail (last ~600 cols) into
                    # 128-col pieces so each piece's DVE+store drain hides
                    # under the next piece's matmuls instead of dangling
                    # after the very last one.
                    mm2_chunks = list(chunks[:-2])
                    for c0l, cwl in chunks[-2:]:
                        mm2_chunks += [
                            (c0l + o, min(128, cwl - o))
                            for o in range(0, cwl, 128)
                        ]
                for ci, (c0, cw) in enumerate(mm2_chunks):
                    ye = y_pool.tile([P, 2, 512], F32, tag="y")
                    for dj in range(2):
                        yps = ps2.tile([P, 512], F32, tag="ps2")
                        for mt in range(MT):
                            nc.tensor.matmul(
                                yps[:, :cw],
                                w2t[:, mt, dj * P : (dj + 1) * P],
                                h_tiles[mt][:, c0 : c0 + cw],
                                start=(mt == 0),
                                stop=(mt == MT - 1),
                            )
                        dt = dtp * 2 + dj
                        nc.vector.tensor_scalar(
                            ye[:, dj, :cw],
                            yps[:, :cw],
                            b2_sb[:, dt : dt + 1],
                            None,
                            op0=ALU.add,
                        )
                        nc.vector.tensor_mul(
                            ye[:, dj, :cw],
                            ye[:, dj, :cw],
                            wb_sb[:, g0 + c0 : g0 + c0 + cw],
                        )
                    # One merged store for both d-tiles, issued from the
                    # ACT queue (idle during mm2) off the SP ring.  In the
                    # final d-pair the loads are done, so alternate with the
                    # idle SP ring to keep store *dispatches* off the
                    # end-of-kernel critical path.
                    if mm2_chunks is not chunks and ci % 2 == 0:
                        store_eng = nc.sync
                    else:
                        store_eng = nc.scalar
                    store_eng.dma_start(
                        yTr[
                            :,
                            dtp * 2 : dtp * 2 + 2,
                            g0 + c0 : g0 + c0 + cw,
                        ],
                        ye[:, :, :cw],
                    )
                if dtp + 2 < DT // 2:
                    w2_tiles[dtp + 2] = issue_w2(dtp + 2)

    nc.compile()
    return nc


_nc_cache = {}


def _get_nc(cap):
    if cap not in _nc_cache:
        _nc_cache[cap] = build_nc(cap)
    return _nc_cache[cap]


def host_route(xf, gate_W):
    """Host gate: top-2 expert indices + normalized combine weights."""
    logits = xf @ gate_W.T.astype(np.float32)
    gmax = logits.max(axis=1, keepdims=True)
    gexp = np.exp(logits - gmax)
    gate = gexp / gexp.sum(axis=1, keepdims=True)
    top2 = np.argpartition(gate, E - 2, axis=1)[:, E - 2 :]
    tw = np.take_along_axis(gate, top2, axis=1)
    tw = tw / (tw.sum(axis=1, keepdims=True) + 1e-9)
    idx, wsel = [], []
    for e in range(E):
        hit = top2 == e
        rows = np.nonzero(hit.any(axis=1))[0]
        w_e = (np.take_along_axis(tw, hit.argmax(axis=1)[:, None], axis=1))[
            rows, 0
        ]
        idx.append(rows)
        wsel.append(w_e.astype(np.float32))
    return idx, wsel


def make_in_maps(xf, W1, b1, W2, b2, idx, wsel, cap):
    in_maps = []
    for e in range(E):
        xg = np.zeros((D, cap), NP_BF16)
        ne = len(idx[e])
        xg[:, :ne] = xf[idx[e]].T.astype(NP_BF16)
        wrow = np.zeros((cap,), NP_BF16)
        wrow[:ne] = wsel[e].astype(NP_BF16)
        wrep = np.ascontiguousarray(np.broadcast_to(wrow, (P, cap)))
        in_maps.append(
            {
                "xgT": xg,
                "w1": np.ascontiguousarray(W1[e]).astype(NP_BF16),
                "w2": np.ascontiguousarray(W2[e]).astype(NP_BF16),
                "b1v": np.ascontiguousarray(b1[e]),
                "b2v": np.ascontiguousarray(b2[e]),
                "wrep": wrep,
            }
        )
    return in_maps


def kernel(**inputs):
    from concourse.bass_utils import run_bass_kernel_spmd

    x = np.asarray(inputs["x"], dtype=np.float32)
    gate_W = np.asarray(inputs["gate_W"], dtype=np.float32)
    W1 = np.asarray(inputs["W1"], dtype=np.float32)
    b1 = np.asarray(inputs["b1"], dtype=np.float32)
    W2 = np.asarray(inputs["W2"], dtype=np.float32)
    b2 = np.asarray(inputs["b2"], dtype=np.float32)

    Bs, Ss, Ds = x.shape
    xf = np.ascontiguousarray(x.reshape(-1, Ds))
    idx, wsel = host_route(xf, gate_W)
    cap = max(P, -(-max(len(i) for i in idx) // 32) * 32)

    nc = _get_nc(cap)
    in_maps = make_in_maps(xf, W1, b1, W2, b2, idx, wsel, cap)
    res = run_bass_kernel_spmd(nc, in_maps, core_ids=list(range(E)))

    out = np.zeros_like(xf)
    for e in range(E):
        yTe = res.results[e]["yT"]  # [D, cap]
        ne = len(idx[e])
        out[idx[e]] += yTe[:, :ne].T
    return out.reshape(Bs, Ss, Ds)
